# revision 1
# baseline (speedup 1.0000x reference)
"""Single-head causal attention (B=8, T=2048, C=384, H=64) on 8 NeuronCores.

Data-parallel over batch: core b computes attention for batch element b.
v3 pipeline (all matmuls bf16, fp32 PSUM):
  - host pre-transposes x -> xT chunks, packs Wqk = [Wq|Wk] per 128-chunk of C
  - QK proj: psum[0:64]=qT, psum[64:128]=kT via packed stationary (12 MMs
    N=512); vT proj via Wv stationary (12 MMs); v[s,h] blocks by PE transpose
  - qk replicated to the other partition half (SBUF->SBUF DMA) so score
    matmuls (contraction H=64) alternate PE row-groups per PSUM bank: same
    bank => same row-group (serialized -- concurrent same-bank matmuls crash
    the runtime), adjacent banks => different row-groups (run concurrently)
  - scores stream into [128,1024] PSUM windows (ring of 3); one ACTIVATE(Exp)
    per window (psum f32 -> sbuf bf16 PT); diagonals masked on GpSimd
  - output transposed: outT[h, t] += v_j[s, 0:65].T @ PT_j[s, t]; the ones
    column 64 gives the softmax denominator in row 64.  Accumulation is
    grouped per (4-strip batch, 512-col chunk) -- consecutive start..stop
    matmul groups (the only accumulation hardware honors) -- drained into an
    f32 SBUF accumulator by DVE copy/add.  Chunk groups unlock progressively
    as their own 4 strips are exp'd, so the tail after the last exp is tiny
  - normalize per quarter: PE-transpose [65,128] blocks back to [t, 65]
    (f32), 128-lane DVE reciprocal of col 64 + scale, DMA out f32
  - all work for window k is emitted at window k+1's flush so the PE queue
    never blocks on ACT; ACT table preloaded via dummy exp during input DMA
"""

import math
import os

import numpy as np
import ml_dtypes

import concourse.bass as bass
import concourse.tile as tile
from concourse import bacc, mybir
from concourse.bass import ds, ts
from concourse.bass_utils import run_bass_kernel_spmd

F32 = mybir.dt.float32
BF16 = mybir.dt.bfloat16

B, T, C, H = 8, 2048, 384, 64
P = 128
NT = T // P          # 16 key/query blocks
NCC = C // P         # 3 contraction chunks
WIN = 1024           # score window columns (2 PSUM banks)
TOTF = NT * (NT + 1) // 2 * P   # total score columns (17408)
SCALE = 1.0 / math.sqrt(float(C))

LAST_RESULT = None
_PROGRAM = None


def _score_chunks():
    """Yield (j, t0, w, fill) for the score chunk stream.

    Strips sequential (j = 0..15), chunks crossing neither a fill-512 (PSUM
    bank) nor a t-512 boundary.
    """
    fill = 0
    for j in range(NT):
        t = P * j
        while t < T:
            w = min(512 - fill % 512, 512 - t % 512, T - t)
            yield (j, t, w, fill)
            t += w
            fill += w


def _emit(tc: tile.TileContext, xT_d, wqk_d, wv_d, mask_d, ident_d,
          out_d, ctx, dbg_d=None):
    nc = tc.nc
    Exp = mybir.ActivationFunctionType.Exp

    sb = ctx.enter_context(tc.tile_pool(name="sb", bufs=1))
    ps = ctx.enter_context(tc.tile_pool(name="ps", bufs=1, space="PSUM"))

    # ---- sbuf tiles -------------------------------------------------------
    wqk_sb = sb.tile([P, NCC, P], BF16, tag="wqk")
    wv_sb = sb.tile([P, NCC, H], BF16, tag="wv")
    mask_sb = sb.tile([P, P], BF16, tag="mask")
    xTt = sb.tile([P, 4, NCC, 512], BF16, tag="xTt")
    qk_nat = sb.tile([P, T], BF16, tag="qk_nat")   # q in rows 0:64, k in 64:128
    qk_swp = sb.tile([P, T], BF16, tag="qk_swp")   # k in rows 0:64, q in 64:128
    vTsb = sb.tile([H, T], BF16, tag="vTsb")
    v_sb = sb.tile([P, NT, H + 1], BF16, tag="v_sb")
    ident = sb.tile([P, P], BF16, tag="ident")
    n_win = (TOTF + WIN - 1) // WIN
    pt_all = sb.tile([P, n_win * WIN], BF16, tag="pt_all")
    outd = sb.tile([H + 1, T], BF16, tag="outd")   # transposed out accumulator
    dum = sb.tile([1, 8], BF16, tag="dum")
    dum2 = sb.tile([1, 8], BF16, tag="dum2")
    warm = sb.tile([P, 512], BF16, tag="warm")

    def xTc(c, t4):
        return xTt[:, t4, c, :]

    # ---- ACT table preload first (1.3us DMA into ACT table RAM, runs
    # while the input DMAs stream) ------------------------------------------
    nc.vector.memset(dum[:], 0.0)
    nc.scalar.activation(dum2[:], dum[:], Exp, scale=SCALE)
    nc.vector.memset(v_sb[:, :, H], 1.0)
    nc.vector.memset(warm[:], 0.0)

    # ---- input DMAs, split across the two DGE queues (sync + scalar).
    # Each x chunk is one contiguous 3KB-per-partition run (small-descriptor
    # destination patterns run the DMA at ~60 GB/s instead of ~350)
    nc.sync.dma_start(wqk_sb[:], wqk_d[:])
    nc.scalar.dma_start(wv_sb[:], wv_d[:])
    nc.sync.dma_start(xTt[:, 0], xT_d[0])
    nc.scalar.dma_start(xTt[:, 1], xT_d[1])
    nc.sync.dma_start(xTt[:, 2], xT_d[2])
    nc.scalar.dma_start(xTt[:, 3], xT_d[3])
    nc.scalar.dma_start(mask_sb[:], mask_d[:])
    nc.scalar.dma_start(ident[:], ident_d[:])

    # PE warm-up while the input DMAs stream: HAM starts throttled at
    # 1.2 GHz and needs ~3.4us of sustained array activity to unthrottle
    wp = ps.tile([P, 512], F32, tag="acc", bufs=4, name="warm_ps")
    for _ in range(11):
        nc.tensor.matmul(wp[:], warm[:, 0:P], warm[:], start=True, stop=True)

    def emit_vtr(j):
        # v block j via PE transpose (XBAR DMA transposes cost ~1.2us each
        # on a DGE queue -- way too slow)
        tr = ps.tile([P, H], BF16, tag="acc", bufs=4, name=f"vtr{j}")
        nc.tensor.transpose(tr[:], vTsb[:, ds(P * j, P)], ident[0:H, 0:H])
        nc.vector.tensor_copy(v_sb[:, j, 0:H], tr[:])

    # ---- projections for one 512-col t-chunk ------------------------------
    def emit_proj(t4):
        w = ps.tile([P, WIN], F32, tag="win", bufs=2, name=f"proj{t4}")
        for c in range(NCC):
            nc.tensor.matmul(
                w[:, 0:512], wqk_sb[:, c, :], xTc(c, t4),
                start=(c == 0), stop=(c == NCC - 1),
            )
        for c in range(NCC):
            nc.tensor.matmul(
                w[0:H, 512:1024], wv_sb[:, c, :], xTc(c, t4),
                start=(c == 0), stop=(c == NCC - 1),
            )
        nc.vector.tensor_copy(qk_nat[:, ts(t4, 512)], w[:, 0:512])
        nc.vector.tensor_copy(vTsb[:, ts(t4, 512)], w[0:H, 512:1024])
        # replicate to the other partition half (k -> low, q -> high)
        nc.sync.dma_start(qk_swp[0:H, ts(t4, 512)], qk_nat[H:P, ts(t4, 512)])
        nc.sync.dma_start(qk_swp[H:P, ts(t4, 512)], qk_nat[0:H, ts(t4, 512)])

    # ---- main loop --------------------------------------------------------
    # score operands by row-group: rows 0:64 = (k from swp, q from nat),
    # rows 64:128 = (k from nat, q from swp)
    qA, kA = qk_nat[0:H, :], qk_swp[0:H, :]
    qB, kB = qk_swp[H:P, :], qk_nat[H:P, :]

    out_v = out_d.rearrange("(g i p) h -> g p i h", p=P, i=4)

    all_chunks = list(_score_chunks())
    # pt layout: strip j occupies pt_all[:, strip_base[j] : +T-128j] contiguous
    strip_base = {}
    for (j, t0, w, fill) in all_chunks:
        if j not in strip_base:
            strip_base[j] = fill

    # outT work units: (batch b of strips 4b..4b+3, 512-col chunk q >= b).
    # Unlock window = when all 4 strips' pt covers t < 512(q+1).
    units = []
    for b in range(4):
        for q in range(b, 4):
            need = max(
                strip_base[j] + 512 * (q + 1) - P * j
                for j in range(4 * b, 4 * b + 4)
            )
            units.append((min((need - 1) // WIN, n_win - 1), b, q))
    units.sort()
    q_parts_done = [0] * 4

    win_tiles = {}
    pending = []              # chunks of the newest un-exped window

    def emit_unit(b, q):
        # one consecutive accumulation group: strips 4b..4b+3 into out cols
        # [512q, 512q+512); strips entering mid-chunk join at partial width
        oa = ps.tile([P, 512], F32, tag="acc", bufs=4, name=f"u{b}_{q}")
        js = list(range(4 * b, 4 * b + 4))
        for n, j in enumerate(js):
            lo = max(512 * q, P * j)
            nc.tensor.matmul(
                oa[0:H + 1, ds(lo - 512 * q, 512 * (q + 1) - lo)],
                v_sb[:, j, 0:H + 1],
                pt_all[:, ds(strip_base[j] + lo - P * j, 512 * (q + 1) - lo)],
                start=(n == 0), stop=(n == len(js) - 1),
                skip_group_check=True,
            )
        if b == 0:
            nc.vector.tensor_copy(outd[0:H + 1, ts(q, 512)], oa[0:H + 1, :])
        else:
            nc.vector.tensor_add(
                outd[0:H + 1, ts(q, 512)], outd[0:H + 1, ts(q, 512)],
                oa[0:H + 1, :],
            )
        q_parts_done[q] += 1
        if q_parts_done[q] == q + 1:
            emit_qnorm(q)

    def emit_qnorm(q):
        # normalize quarter q: PE-transpose each 128-block back to [t, 65]
        # (f32), then 128-lane reciprocal + scale on DVE (a single-partition
        # reciprocal on the denominator row costs 3.3us -- never do that)
        outf = sb.tile([P, 4, H], F32, tag="outf", bufs=2, name=f"outf{q}")
        r = sb.tile([P, 4], F32, tag="recip", bufs=2, name=f"recip{q}")
        for bb in range(4):
            tr = ps.tile([P, H + 1], BF16, tag="acc", bufs=4,
                         name=f"otr{q}_{bb}")
            nc.tensor.transpose(
                tr[:], outd[:, ds(512 * q + P * bb, P)],
                ident[0:H + 1, 0:H + 1]
            )
            nc.vector.reciprocal(r[:, ds(bb, 1)], tr[:, H:H + 1])
            nc.vector.tensor_scalar_mul(outf[:, bb, :], tr[:, 0:H],
                                        r[:, ds(bb, 1)])
        nc.sync.dma_start(out_v[q], outf[:])

    def flush(wid):
        # exp the filled window; then (while ACT runs) masks, v transposes,
        # and any outT unit groups whose strips are now all exp'd
        nonlocal pending
        if not pending:
            return
        wt, fill = win_tiles.pop(wid)
        pt0 = wid * WIN
        nc.scalar.activation(pt_all[:, ds(pt0, fill)], wt[:, 0:fill], Exp,
                             scale=SCALE)
        for (j, t0, w, fpos) in pending:
            pt_off = pt0 + fpos
            # mask any part of this chunk inside the strip's diagonal block
            dlo, dhi = P * j, P * j + P
            mlo, mhi = max(t0, dlo), min(t0 + w, dhi)
            if mlo < mhi:
                nc.gpsimd.tensor_mul(
                    pt_all[:, ds(pt_off + (mlo - t0), mhi - mlo)],
                    pt_all[:, ds(pt_off + (mlo - t0), mhi - mlo)],
                    mask_sb[:, ds(mlo - dlo, mhi - mlo)],
                )
        pending = []

    emit_proj(0)
    emit_proj(1)
    emit_proj(2)
    emit_proj(3)
    cur_wid = 0
    for (j, t0, w, fill) in all_chunks:
        wid, fpos = fill // WIN, fill % WIN
        if wid != cur_wid:
            flush(cur_wid)
            cur_wid = wid
        if fpos == 0:
            wt = ps.tile([P, WIN], F32, tag="win", bufs=2, name=f"win{wid}")
            win_tiles[wid] = (wt, 0)
        wt, wfill = win_tiles[wid]
        assert wfill == fpos, (wfill, fpos)
        rg = (fill // 512) % 2
        stat = kA if rg == 0 else kB
        mov = qA if rg == 0 else qB
        nc.tensor.matmul(
            wt[:, ds(fpos, w)],
            stat[:, ds(P * j, P)],
            mov[:, ds(t0, w)],
            start=True, stop=True,
        )
        win_tiles[wid] = (wt, wfill + w)
        pending.append((j, t0, w, fpos))
    flush(cur_wid)
    # lower-priority filler work: the Tile scheduler slots these into PE
    # gaps as their dependencies (projections / exps / masks) resolve
    for j in range(NT):
        emit_vtr(j)
    for (_w, b, q) in units:
        emit_unit(b, q)
    if dbg_d is not None:
        nc.sync.dma_start(dbg_d[:, 0:NT * (H + 1)],
                          v_sb.rearrange("p j h -> p (j h)"))
        nc.sync.dma_start(dbg_d[:, 2048:2048 + 4096],
                          pt_all[:, 0:4096])


def _build_program(num_devices=B, debug_out=False):
    nc = bacc.Bacc("TRN2", target_bir_lowering=False, debug=False,
                   num_devices=num_devices)
    xT_d = nc.dram_tensor("xT", [4, P, NCC, 512], BF16,
                          kind="ExternalInput").ap()
    wqk_d = nc.dram_tensor("wqk", [P, NCC, P], BF16, kind="ExternalInput").ap()
    wv_d = nc.dram_tensor("wv", [P, NCC, H], BF16, kind="ExternalInput").ap()
    mask_d = nc.dram_tensor("mask", [P, P], BF16, kind="ExternalInput").ap()
    ident_d = nc.dram_tensor("ident", [P, P], BF16, kind="ExternalInput").ap()
    out_d = nc.dram_tensor("out", [T, H], F32, kind="ExternalOutput").ap()
    dbg_d = None
    if debug_out:
        dbg_d = nc.dram_tensor("dbg", [P, 8192], BF16,
                               kind="ExternalOutput").ap()
    from contextlib import ExitStack

    with tile.TileContext(nc) as tc:
        with ExitStack() as ctx:
            _emit(tc, xT_d, wqk_d, wv_d, mask_d, ident_d,
                  out_d, ctx, dbg_d=dbg_d)
    nc.compile()
    return nc


def _host_inputs(x, Wq, Wk, Wv):
    bf = ml_dtypes.bfloat16
    xT = np.ascontiguousarray(np.transpose(x, (0, 2, 1))).astype(bf)
    Bn = x.shape[0]
    # xT: [t4, 128, c, 512] -- one contiguous run per (partition, t4)
    xTr = xT.reshape(Bn, NCC, P, 4, 512)
    xTn = np.ascontiguousarray(xTr.transpose(0, 3, 2, 1, 4))
    wqk = np.concatenate([Wq, Wk], axis=1).reshape(NCC, P, 2 * H)
    wqk = np.ascontiguousarray(np.transpose(wqk, (1, 0, 2))).astype(bf)
    wv = np.ascontiguousarray(
        np.transpose(Wv.reshape(NCC, P, H), (1, 0, 2))
    ).astype(bf)
    # mask[s, t] = 1 where s <= t (transposed-causal diagonal block)
    mask = np.triu(np.ones((P, P), dtype=np.float32)).astype(bf)
    identity = np.eye(P, dtype=np.float32).astype(bf)
    return xTn, wqk, wv, mask, identity


def kernel(x, Wq, Wk, Wv):
    global LAST_RESULT, _PROGRAM
    assert x.shape == (B, T, C), x.shape
    if _PROGRAM is None:
        _PROGRAM = _build_program()
    nc = _PROGRAM

    xTn, wqk, wv, mask, identity = _host_inputs(x, Wq, Wk, Wv)
    in_maps = [
        {"xT": xTn[b], "wqk": wqk, "wv": wv, "mask": mask, "ident": identity}
        for b in range(B)
    ]
    trace = bool(int(os.environ.get("KERNEL_TRACE", "0")))
    kw = {}
    td = os.environ.get("KERNEL_TRACE_DIR")
    if td:
        kw["tmpdir"] = td
    LAST_RESULT = run_bass_kernel_spmd(
        nc, in_maps, list(range(B)), trace=trace, **kw
    )
    out = np.stack([LAST_RESULT.results[b]["out"] for b in range(B)], axis=0)
    return out.astype(np.float32)



# revision 5
# speedup vs baseline: 1.0772x; 1.0772x over previous
"""Single-head causal attention (B=8, T=2048, C=384, H=64) on 8 NeuronCores.

Data-parallel over batch: core b computes attention for batch element b.
v4 pipeline (all matmuls bf16, fp32 PSUM):
  - host pre-transposes x -> xT chunks, packs Wqk = [Wq|Wk] per 128-chunk of C
  - inputs stream on FOUR DMA queues (sync/scalar/vector/gpsimd) issued
    first thing, one x chunk each, so projections start ~4us earlier
  - QK proj: psum[0:64]=qT, psum[64:128]=kT via packed stationary (12 MMs
    N=512, into the score-window PSUM pool); vT proj via Wv stationary
    (into the acc pool); v[s,h] blocks by PE transpose
  - qk replicated to the other partition half (SBUF->SBUF DMA, split over
    sync+gpsimd queues) so score matmuls (contraction H=64) alternate PE
    row-groups per PSUM bank: same bank => same row-group, adjacent banks
    => different row-groups (run concurrently)
  - scores stream into PSUM windows from a ring of 2 [128,1536] tiles
    (3 banks each); ONE ACTIVATE(Exp) per window.  Window sizes are
    512, 1536 x10, 1024, 512: the small first window starts the serial
    ACT exp chain (the critical path, ~18us) as early as possible and the
    small last window shortens the post-last-exp tail.  13 ACTIVATEs
    instead of 17 also cuts ACT fixed overhead.  Diagonal blocks are
    masked on GpSimd after each exp
  - output transposed: outT[h, t] += v_j[s, 0:65].T @ PT_j[s, t]; the ones
    column 64 gives the softmax denominator in row 64.  Accumulation is
    grouped per (4-strip batch, 512-col chunk) -- consecutive start..stop
    matmul groups -- drained into a bf16 SBUF accumulator by DVE copy/add.
    Units are emitted two windows behind the score stream so the PE never
    stalls on an in-flight exp; v-projections and v-transposes are slotted
    as PE filler between early windows
  - NO on-device normalize: the [65, T] bf16 accumulator is DMA'd out
    per 512-col quarter as soon as complete (1KB contiguous runs per
    partition); the host does out[t,h] = num[h,t]/den[t] and transposes.
    This kills the PE transposes + single-partition reciprocals + the
    256B-descriptor output DMA of v3
  - ACT table preloaded via dummy exp during input DMA; PE warm-up matmuls
    run while the input DMAs stream (HAM unthrottles after ~3.4us busy)
"""

import bisect
import math
import os

import numpy as np
import ml_dtypes

import concourse.bass as bass
import concourse.tile as tile
from concourse import bacc, mybir
from concourse.bass import ds, ts
from concourse.bass_utils import run_bass_kernel_spmd

F32 = mybir.dt.float32
BF16 = mybir.dt.bfloat16

B, T, C, H = 8, 2048, 384, 64
P = 128
NT = T // P          # 16 key/query blocks
NCC = C // P         # 3 contraction chunks
WIN = 1536           # max score window columns (3 PSUM banks)
TOTF = NT * (NT + 1) // 2 * P   # total score columns (17408)
SCALE = 1.0 / math.sqrt(float(C))

# window boundaries in fill space: 512, 1536 x10, 1024, 512
WBOUND = [0, 512]
while WBOUND[-1] + WIN <= TOTF - WIN:
    WBOUND.append(WBOUND[-1] + WIN)
WBOUND += [TOTF - 512, TOTF]
N_WIN = len(WBOUND) - 1
assert WBOUND[-2] - WBOUND[-3] in (512, 1024, WIN), WBOUND

LAST_RESULT = None
_PROGRAM = None


def _wid_of(fill):
    return bisect.bisect_right(WBOUND, fill) - 1


def _score_chunks():
    """Yield (j, t0, w, fill) for the score chunk stream.

    Strips sequential (j = 0..15); chunks break only at fill-512 (PSUM
    bank) boundaries, not at t-512 boundaries (the moving operand may
    span them freely).  Window boundaries are 512-multiples so chunks
    never straddle windows.
    """
    fill = 0
    for j in range(NT):
        t = P * j
        while t < T:
            w = min(512 - fill % 512, T - t)
            yield (j, t, w, fill)
            t += w
            fill += w


def _emit(tc: tile.TileContext, xT_d, wqk_d, wv_d, mask_d, ident_d,
          out_d, ctx, dbg_d=None):
    nc = tc.nc
    Exp = mybir.ActivationFunctionType.Exp

    sb = ctx.enter_context(tc.tile_pool(name="sb", bufs=1))
    ps = ctx.enter_context(tc.tile_pool(name="ps", bufs=1, space="PSUM"))

    # ---- sbuf tiles -------------------------------------------------------
    wqk_sb = sb.tile([P, NCC, P], BF16, tag="wqk")
    wv_sb = sb.tile([P, NCC, H], BF16, tag="wv")
    mask_sb = sb.tile([P, P], BF16, tag="mask")
    xTt = sb.tile([P, 4, NCC, 512], BF16, tag="xTt")
    qk_nat = sb.tile([P, T], BF16, tag="qk_nat")   # q in rows 0:64, k in 64:128
    qk_swp = sb.tile([P, T], BF16, tag="qk_swp")   # k in rows 0:64, q in 64:128
    vTsb = sb.tile([H, T], BF16, tag="vTsb")
    v_sb = sb.tile([P, NT, H + 1], BF16, tag="v_sb")
    ident = sb.tile([P, P], BF16, tag="ident")
    pt_all = sb.tile([P, TOTF], BF16, tag="pt_all")
    outd = sb.tile([H + 1, T], BF16, tag="outd")   # transposed out accumulator
    dum = sb.tile([1, 8], BF16, tag="dum")
    dum2 = sb.tile([1, 8], BF16, tag="dum2")
    warm = sb.tile([P, 512], BF16, tag="warm")

    def xTc(c, t4):
        return xTt[:, t4, c, :]

    # ---- memsets that gate early work, then input DMAs (one x chunk per
    # DGE queue so they stream in parallel) ---------------------------------
    nc.vector.memset(warm[:], 0.0)           # gates PE warm-up
    nc.gpsimd.memset(dum[:], 0.0)            # gates ACT table preload
    nc.sync.dma_start(xTt[:, 0], xT_d[0])
    nc.scalar.dma_start(xTt[:, 1], xT_d[1])
    nc.gpsimd.dma_start(xTt[:, 2], xT_d[2])
    nc.sync.dma_start(xTt[:, 3], xT_d[3])
    nc.scalar.dma_start(wqk_sb[:], wqk_d[:])
    nc.gpsimd.dma_start(wv_sb[:], wv_d[:])
    nc.gpsimd.dma_start(mask_sb[:], mask_d[:])
    nc.gpsimd.dma_start(ident[:], ident_d[:])
    nc.vector.memset(v_sb[:, :, H], 1.0)

    # ACT table preload: 1.3us DMA into ACT table RAM, runs while the
    # input DMAs stream
    nc.scalar.activation(dum2[:], dum[:], Exp, scale=SCALE)

    # PE warm-up while the input DMAs stream: HAM starts throttled at
    # 1.2 GHz and needs ~3.4us of sustained array activity to unthrottle
    wp = ps.tile([P, 512], F32, tag="acc", bufs=2, name="warm_ps")
    for _ in range(6):
        nc.tensor.matmul(wp[:], warm[:, 0:P], warm[:], start=True, stop=True)

    # ---- projections ------------------------------------------------------
    def emit_projqk(t4):
        # packed q|k projection for one 512-col t-chunk, into the win pool
        # (only [0:512] of the 1536-wide slot is used)
        w = ps.tile([P, WIN], F32, tag="win", bufs=2, name=f"projqk{t4}")
        for c in range(NCC):
            nc.tensor.matmul(
                w[:, 0:512], wqk_sb[:, c, :], xTc(c, t4),
                start=(c == 0), stop=(c == NCC - 1),
            )
        nc.vector.tensor_copy(qk_nat[:, ts(t4, 512)], w[:, 0:512])
        # replicate to the other partition half (k -> low, q -> high);
        # alternate queues so the 8 issue latencies overlap
        eng = nc.sync if t4 % 2 == 0 else nc.gpsimd
        eng.dma_start(qk_swp[0:H, ts(t4, 512)], qk_nat[H:P, ts(t4, 512)])
        eng.dma_start(qk_swp[H:P, ts(t4, 512)], qk_nat[0:H, ts(t4, 512)])

    def emit_projv(t4):
        w = ps.tile([P, 512], F32, tag="acc", bufs=2, name=f"projv{t4}")
        for c in range(NCC):
            nc.tensor.matmul(
                w[0:H, :], wv_sb[:, c, :], xTc(c, t4),
                start=(c == 0), stop=(c == NCC - 1),
            )
        nc.vector.tensor_copy(vTsb[:, ts(t4, 512)], w[0:H, :])

    def emit_vtr(j):
        # v block j via PE transpose (XBAR DMA transposes cost ~1.2us each
        # on a DGE queue -- way too slow)
        tr = ps.tile([P, H], BF16, tag="acc", bufs=2, name=f"vtr{j}")
        nc.tensor.transpose(tr[:], vTsb[:, ds(P * j, P)], ident[0:H, 0:H])
        nc.vector.tensor_copy(v_sb[:, j, 0:H], tr[:])

    # ---- main loop --------------------------------------------------------
    # score operands by row-group: rows 0:64 = (k from swp, q from nat),
    # rows 64:128 = (k from nat, q from swp)
    qA, kA = qk_nat[0:H, :], qk_swp[0:H, :]
    qB, kB = qk_swp[H:P, :], qk_nat[H:P, :]

    all_chunks = list(_score_chunks())
    # pt layout: strip j occupies pt_all[:, strip_base[j] : +T-128j] contiguous
    strip_base = {}
    for (j, t0, w, fill) in all_chunks:
        if j not in strip_base:
            strip_base[j] = fill

    # outT work units: (batch b of strips 4b..4b+3, 512-col chunk q >= b).
    # Unlock window = when all 4 strips' pt covers t < 512(q+1).
    units = []
    for b in range(4):
        for q in range(b, 4):
            need = max(
                strip_base[j] + 512 * (q + 1) - P * j
                for j in range(4 * b, 4 * b + 4)
            )
            units.append((_wid_of(need - 1), b, q))
    units.sort()
    q_parts_done = [0] * 4

    win_tiles = {}
    pending = []              # chunks of the newest un-exped window

    def emit_unit(b, q):
        # one consecutive accumulation group: strips 4b..4b+3 into out cols
        # [512q, 512q+512); strips entering mid-chunk join at partial width
        oa = ps.tile([P, 512], F32, tag="acc", bufs=2, name=f"u{b}_{q}")
        js = list(range(4 * b, 4 * b + 4))
        for n, j in enumerate(js):
            lo = max(512 * q, P * j)
            nc.tensor.matmul(
                oa[0:H + 1, ds(lo - 512 * q, 512 * (q + 1) - lo)],
                v_sb[:, j, 0:H + 1],
                pt_all[:, ds(strip_base[j] + lo - P * j, 512 * (q + 1) - lo)],
                start=(n == 0), stop=(n == len(js) - 1),
                skip_group_check=True,
            )
        if b == 0:
            nc.vector.tensor_copy(outd[0:H + 1, ts(q, 512)], oa[0:H + 1, :])
        else:
            nc.vector.tensor_add(
                outd[0:H + 1, ts(q, 512)], outd[0:H + 1, ts(q, 512)],
                oa[0:H + 1, :],
            )
        q_parts_done[q] += 1
        if q_parts_done[q] == q + 1:
            # quarter complete: ship it (bf16, 1KB contiguous per partition;
            # host divides by the denominator row and transposes)
            nc.sync.dma_start(out_d[:, ts(q, 512)], outd[:, ts(q, 512)])

    def flush(wid):
        # exp the filled window; then (while ACT runs) masks on GpSimd
        nonlocal pending
        if not pending:
            return
        wt, fill = win_tiles.pop(wid)
        assert fill == WBOUND[wid + 1] - WBOUND[wid], (wid, fill)
        pt0 = WBOUND[wid]
        nc.scalar.activation(pt_all[:, ds(pt0, fill)], wt[:, 0:fill], Exp,
                             scale=SCALE)
        for (j, t0, w, fpos) in pending:
            pt_off = pt0 + fpos
            # mask any part of this chunk inside the strip's diagonal block
            dlo, dhi = P * j, P * j + P
            mlo, mhi = max(t0, dlo), min(t0 + w, dhi)
            if mlo < mhi:
                nc.gpsimd.tensor_mul(
                    pt_all[:, ds(pt_off + (mlo - t0), mhi - mlo)],
                    pt_all[:, ds(pt_off + (mlo - t0), mhi - mlo)],
                    mask_sb[:, ds(mlo - dlo, mhi - mlo)],
                )
        pending = []

    emit_projqk(0)
    emit_projqk(1)
    emit_projqk(2)
    emit_projqk(3)
    emit_projv(0)

    # PE filler work slotted after specific windows: v-projections early
    # (they gate the v transposes), then vtr batches just before the units
    # that consume them unlock
    fillers = {
        0: [lambda: emit_projv(1)],
        1: [lambda: emit_projv(2)],
        2: [lambda: emit_projv(3)],
        3: [lambda j=j: emit_vtr(j) for j in range(0, 4)],
        4: [lambda j=j: emit_vtr(j) for j in range(4, 8)],
        5: [lambda j=j: emit_vtr(j) for j in range(8, 12)],
        6: [lambda j=j: emit_vtr(j) for j in range(12, 16)],
    }

    unit_i = 0

    def emit_ready_units(through_wid):
        # emit units whose unlock window has already been exp'd (two
        # windows behind the score stream, so the PE never waits on an
        # in-flight exp)
        nonlocal unit_i
        while unit_i < len(units) and units[unit_i][0] <= through_wid:
            _w, b, q = units[unit_i]
            emit_unit(b, q)
            unit_i += 1

    cur_wid = 0
    for (j, t0, w, fill) in all_chunks:
        wid, fpos = _wid_of(fill), fill - WBOUND[_wid_of(fill)]
        if wid != cur_wid:
            flush(cur_wid)
            for f in fillers.get(cur_wid, ()):
                f()
            emit_ready_units(cur_wid - 1)
            cur_wid = wid
        if fpos == 0:
            wt = ps.tile([P, WIN], F32, tag="win", bufs=2, name=f"win{wid}")
            win_tiles[wid] = (wt, 0)
        wt, wfill = win_tiles[wid]
        assert wfill == fpos, (wfill, fpos)
        rg = (fill // 512) % 2
        stat = kA if rg == 0 else kB
        mov = qA if rg == 0 else qB
        nc.tensor.matmul(
            wt[:, ds(fpos, w)],
            stat[:, ds(P * j, P)],
            mov[:, ds(t0, w)],
            start=True, stop=True,
        )
        win_tiles[wid] = (wt, wfill + w)
        pending.append((j, t0, w, fpos))
    flush(cur_wid)
    emit_ready_units(N_WIN)
    assert unit_i == len(units), (unit_i, len(units))
    if dbg_d is not None:
        nc.sync.dma_start(dbg_d[:, 0:NT * (H + 1)],
                          v_sb.rearrange("p j h -> p (j h)"))
        nc.sync.dma_start(dbg_d[:, 2048:2048 + 4096],
                          pt_all[:, 0:4096])


def _build_program(num_devices=B, debug_out=False):
    nc = bacc.Bacc("TRN2", target_bir_lowering=False, debug=False,
                   num_devices=num_devices)
    xT_d = nc.dram_tensor("xT", [4, P, NCC, 512], BF16,
                          kind="ExternalInput").ap()
    wqk_d = nc.dram_tensor("wqk", [P, NCC, P], BF16, kind="ExternalInput").ap()
    wv_d = nc.dram_tensor("wv", [P, NCC, H], BF16, kind="ExternalInput").ap()
    mask_d = nc.dram_tensor("mask", [P, P], BF16, kind="ExternalInput").ap()
    ident_d = nc.dram_tensor("ident", [P, P], BF16, kind="ExternalInput").ap()
    out_d = nc.dram_tensor("out", [H + 1, T], BF16, kind="ExternalOutput").ap()
    dbg_d = None
    if debug_out:
        dbg_d = nc.dram_tensor("dbg", [P, 8192], BF16,
                               kind="ExternalOutput").ap()
    from contextlib import ExitStack

    with tile.TileContext(nc) as tc:
        with ExitStack() as ctx:
            _emit(tc, xT_d, wqk_d, wv_d, mask_d, ident_d,
                  out_d, ctx, dbg_d=dbg_d)
    nc.compile()
    return nc


def _host_inputs(x, Wq, Wk, Wv):
    bf = ml_dtypes.bfloat16
    xT = np.ascontiguousarray(np.transpose(x, (0, 2, 1))).astype(bf)
    Bn = x.shape[0]
    # xT: [t4, 128, c, 512] -- one contiguous run per (partition, t4)
    xTr = xT.reshape(Bn, NCC, P, 4, 512)
    xTn = np.ascontiguousarray(xTr.transpose(0, 3, 2, 1, 4))
    wqk = np.concatenate([Wq, Wk], axis=1).reshape(NCC, P, 2 * H)
    wqk = np.ascontiguousarray(np.transpose(wqk, (1, 0, 2))).astype(bf)
    wv = np.ascontiguousarray(
        np.transpose(Wv.reshape(NCC, P, H), (1, 0, 2))
    ).astype(bf)
    # mask[s, t] = 1 where s <= t (transposed-causal diagonal block)
    mask = np.triu(np.ones((P, P), dtype=np.float32)).astype(bf)
    identity = np.eye(P, dtype=np.float32).astype(bf)
    return xTn, wqk, wv, mask, identity


def kernel(x, Wq, Wk, Wv):
    global LAST_RESULT, _PROGRAM
    assert x.shape == (B, T, C), x.shape
    if _PROGRAM is None:
        _PROGRAM = _build_program()
    nc = _PROGRAM

    xTn, wqk, wv, mask, identity = _host_inputs(x, Wq, Wk, Wv)
    in_maps = [
        {"xT": xTn[b], "wqk": wqk, "wv": wv, "mask": mask, "ident": identity}
        for b in range(B)
    ]
    trace = bool(int(os.environ.get("KERNEL_TRACE", "0")))
    kw = {}
    td = os.environ.get("KERNEL_TRACE_DIR")
    if td:
        kw["tmpdir"] = td
    LAST_RESULT = run_bass_kernel_spmd(
        nc, in_maps, list(range(B)), trace=trace, **kw
    )
    out = np.empty((B, T, H), dtype=np.float32)
    for b in range(B):
        acc = LAST_RESULT.results[b]["out"].astype(np.float32)  # [65, T]
        out[b] = (acc[0:H] / acc[H:H + 1]).T
    return out


# revision 6
# speedup vs baseline: 1.0987x; 1.0200x over previous
"""Single-head causal attention (B=8, T=2048, C=384, H=64) on 8 NeuronCores.

Data-parallel over batch: core b computes attention for batch element b.
v5 pipeline (all matmuls bf16, fp32 PSUM):
  - host pre-transposes x -> xT chunks, packs Wqk = [Wq|Wk] per 128-chunk of C
  - x streams on both HW DGE queues (sync+scalar), each 512-col t-chunk
    split in half across them so chunk 0 lands ~1us after first byte;
    small weights ride the gpsimd software queue (not urgent)
  - QK proj: psum[0:64]=qT, psum[64:128]=kT via packed stationary (12 MMs
    N=512); vT proj via Wv stationary; v[s,h] blocks by PE transpose.
    Proj/v/vtr/unit PSUM comes from a 2-buf 1-bank "acc" pool; score
    windows + PE warmup from a 2-buf 3-bank "win" pool
  - qk replicated to the other partition half (SBUF->SBUF DMAs, all on the
    sync queue, k-half first) so score matmuls (contraction H=64) alternate
    PE row-groups per PSUM bank (same bank => same row-group, adjacent
    banks => different row-groups)
  - score stream is COLUMN-CHUNK-MAJOR: region qc covers t in
    [512qc, 512qc+512) for all strips j <= 4qc+3, strips in order inside.
    Region qc only needs x chunks <= qc, so the serial ACT exp chain (the
    ~18us critical path) starts as soon as chunk 0 lands and never starves
    waiting for later x chunks.  It also spreads the out-unit unlocks
    evenly across windows (strip-major bunched them at 3 points)
  - windows sized 512, 1536 x10, 1024, 512 (ring of 2 [128,1536] PSUM
    tiles): small first window starts the exp chain early, small last
    window shortens the post-last-exp tail; 13 ACTIVATEs total
  - output transposed: outT[h, t] += v_j[s, 0:65].T @ PT_j[s, t]; the ones
    column 64 gives the softmax denominator in row 64.  Units (4-strip
    batch, 512-col chunk) are consecutive start..stop accumulation groups
    drained into a bf16 SBUF accumulator by DVE copy/add; units are
    emitted two windows behind the score stream, v-projections and
    v-transposes slotted as PE filler between early windows
  - NO on-device normalize: the [65, T] bf16 accumulator is DMA'd out
    per 512-col quarter as soon as complete (1KB contiguous runs per
    partition); the host does out[t,h] = num[h,t]/den[t] and transposes
  - ACT table preloaded via dummy exp during input DMA; PE warm-up matmuls
    run while the input DMAs stream (HAM unthrottles after ~3.4us busy)
"""

import bisect
import math
import os

import numpy as np
import ml_dtypes

import concourse.bass as bass
import concourse.tile as tile
from concourse import bacc, mybir
from concourse.bass import ds, ts
from concourse.bass_utils import run_bass_kernel_spmd

F32 = mybir.dt.float32
BF16 = mybir.dt.bfloat16

B, T, C, H = 8, 2048, 384, 64
P = 128
NT = T // P          # 16 key/query blocks
NCC = C // P         # 3 contraction chunks
WIN = 1536           # max score window columns (3 PSUM banks)
TOTF = NT * (NT + 1) // 2 * P   # total score columns (17408)
SCALE = 1.0 / math.sqrt(float(C))

# window boundaries in fill space: 512, 1536 x10, 1024, 512
WBOUND = [0, 512]
while WBOUND[-1] + WIN <= TOTF - WIN:
    WBOUND.append(WBOUND[-1] + WIN)
WBOUND += [TOTF - 512, TOTF]
N_WIN = len(WBOUND) - 1

LAST_RESULT = None
_PROGRAM = None


def _wid_of(fill):
    return bisect.bisect_right(WBOUND, fill) - 1


def _score_chunks():
    """Yield (j, t0, w, fill) for the column-chunk-major score stream.

    Region qc = t in [512qc, 512(qc+1)), strips j = 0..4qc+3 in order
    (clipped to t >= 128j).  Chunks break at fill-512 (PSUM bank)
    boundaries; window boundaries are 512-multiples so chunks never
    straddle windows.
    """
    fill = 0
    for qc in range(4):
        for j in range(4 * qc + 4):
            t = max(P * j, 512 * qc)
            t_end = 512 * (qc + 1)
            while t < t_end:
                w = min(512 - fill % 512, t_end - t)
                yield (j, t, w, fill)
                t += w
                fill += w


def _emit(tc: tile.TileContext, xT_d, wqk_d, wv_d, mask_d, ident_d,
          out_d, ctx, dbg_d=None):
    nc = tc.nc
    Exp = mybir.ActivationFunctionType.Exp

    sb = ctx.enter_context(tc.tile_pool(name="sb", bufs=1))
    ps = ctx.enter_context(tc.tile_pool(name="ps", bufs=1, space="PSUM"))

    # ---- sbuf tiles -------------------------------------------------------
    wqk_sb = sb.tile([P, NCC, P], BF16, tag="wqk")
    wv_sb = sb.tile([P, NCC, H], BF16, tag="wv")
    mask_sb = sb.tile([P, P], BF16, tag="mask")
    xTt = sb.tile([P, 4, NCC, 512], BF16, tag="xTt")
    qk_nat = sb.tile([P, T], BF16, tag="qk_nat")   # q in rows 0:64, k in 64:128
    qk_swp = sb.tile([P, T], BF16, tag="qk_swp")   # k in rows 0:64, q in 64:128
    vTsb = sb.tile([H, T], BF16, tag="vTsb")
    v_sb = sb.tile([P, NT, H + 1], BF16, tag="v_sb")
    ident = sb.tile([P, P], BF16, tag="ident")
    pt_all = sb.tile([P, TOTF], BF16, tag="pt_all")
    outd = sb.tile([H + 1, T], BF16, tag="outd")   # transposed out accumulator
    dum = sb.tile([1, 8], BF16, tag="dum")
    dum2 = sb.tile([1, 8], BF16, tag="dum2")
    warm = sb.tile([P, 512], BF16, tag="warm")

    def xTc(c, t4):
        return xTt[:, t4, c, :]

    # ---- memsets that gate early work on gpsimd (it exits the preamble
    # barrier first), then input DMAs --------------------------------------
    nc.gpsimd.memset(warm[:], 0.0)           # gates PE warm-up
    nc.gpsimd.memset(dum[:], 0.0)            # gates ACT table preload
    # x chunks split in half across the two HW DGE queues
    for t4 in range(4):
        nc.sync.dma_start(xTt[0:H, t4], xT_d[t4, 0:H])
        if t4 == 0:
            nc.scalar.dma_start(wqk_sb[:], wqk_d[:])
        nc.scalar.dma_start(xTt[H:P, t4], xT_d[t4, H:P])
    nc.gpsimd.dma_start(wv_sb[:], wv_d[:])
    nc.gpsimd.dma_start(mask_sb[:], mask_d[:])
    nc.gpsimd.dma_start(ident[:], ident_d[:])
    nc.vector.memset(v_sb[:, :, H], 1.0)

    # ACT table preload: 1.3us DMA into ACT table RAM, runs while the
    # input DMAs stream (after the scalar-queue dma issues so it doesn't
    # delay them)
    nc.scalar.activation(dum2[:], dum[:], Exp, scale=SCALE)

    # PE warm-up while the input DMAs stream: HAM starts throttled at
    # 1.2 GHz and needs ~3.4us of sustained array activity to unthrottle
    wp = ps.tile([P, WIN], F32, tag="win", bufs=2, name="warm_ps")
    for _ in range(4):
        nc.tensor.matmul(wp[:, 0:512], warm[:, 0:P], warm[:],
                         start=True, stop=True)

    # ---- projections ------------------------------------------------------
    def emit_projqk(t4):
        # packed q|k projection for one 512-col t-chunk
        w = ps.tile([P, 512], F32, tag="acc", bufs=2, name=f"projqk{t4}")
        for c in range(NCC):
            nc.tensor.matmul(
                w[:, :], wqk_sb[:, c, :], xTc(c, t4),
                start=(c == 0), stop=(c == NCC - 1),
            )
        nc.vector.tensor_copy(qk_nat[:, ts(t4, 512)], w[:, :])
        # replicate to the other partition half, k-half first (window 0
        # only needs the k swap of chunk 0); all on the sync queue -- a
        # dma_start's sem wait stalls its issuing engine, and scalar must
        # stay free for the exp chain
        nc.sync.dma_start(qk_swp[0:H, ts(t4, 512)], qk_nat[H:P, ts(t4, 512)])
        nc.sync.dma_start(qk_swp[H:P, ts(t4, 512)], qk_nat[0:H, ts(t4, 512)])

    def emit_projv(t4):
        w = ps.tile([P, 512], F32, tag="acc", bufs=2, name=f"projv{t4}")
        for c in range(NCC):
            nc.tensor.matmul(
                w[0:H, :], wv_sb[:, c, :], xTc(c, t4),
                start=(c == 0), stop=(c == NCC - 1),
            )
        nc.vector.tensor_copy(vTsb[:, ts(t4, 512)], w[0:H, :])

    def emit_vtr(j):
        # v block j via PE transpose (XBAR DMA transposes cost ~1.2us each
        # on a DGE queue -- way too slow)
        tr = ps.tile([P, H], BF16, tag="acc", bufs=2, name=f"vtr{j}")
        nc.tensor.transpose(tr[:], vTsb[:, ds(P * j, P)], ident[0:H, 0:H])
        nc.vector.tensor_copy(v_sb[:, j, 0:H], tr[:])

    # ---- main loop --------------------------------------------------------
    # score operands by row-group: rows 0:64 = (k from swp, q from nat),
    # rows 64:128 = (k from nat, q from swp)
    qA, kA = qk_nat[0:H, :], qk_swp[0:H, :]
    qB, kB = qk_swp[H:P, :], qk_nat[H:P, :]

    all_chunks = list(_score_chunks())
    # pt layout: (strip j, col-chunk q) occupies pt_all starting at
    # pt_base[(j, q)] (contiguous within the pair), first col max(128j,512q)
    pt_base, pt_end = {}, {}
    for (j, t0, w, fill) in all_chunks:
        q = t0 // 512
        pt_base.setdefault((j, q), fill)
        pt_end[(j, q)] = fill + w

    # outT work units: (batch b of strips 4b..4b+3, 512-col chunk q >= b).
    # Unlock window = when the last strip of the batch has its chunk-q
    # scores exp'd (strips are emitted in order within a region)
    units = []
    for b in range(4):
        for q in range(b, 4):
            need = max(pt_end[(j, q)] for j in range(4 * b, 4 * b + 4))
            units.append((_wid_of(need - 1), b, q))
    units.sort()
    q_parts_done = [0] * 4

    win_tiles = {}
    pending = []              # chunks of the newest un-exped window

    def emit_unit(b, q):
        # one consecutive accumulation group: strips 4b..4b+3 into out cols
        # [512q, 512q+512); strips entering mid-chunk join at partial width
        oa = ps.tile([P, 512], F32, tag="acc", bufs=2, name=f"u{b}_{q}")
        js = list(range(4 * b, 4 * b + 4))
        for n, j in enumerate(js):
            lo = max(512 * q, P * j)
            nc.tensor.matmul(
                oa[0:H + 1, ds(lo - 512 * q, 512 * (q + 1) - lo)],
                v_sb[:, j, 0:H + 1],
                pt_all[:, ds(pt_base[(j, q)], 512 * (q + 1) - lo)],
                start=(n == 0), stop=(n == len(js) - 1),
                skip_group_check=True,
            )
        if b == 0:
            nc.vector.tensor_copy(outd[0:H + 1, ts(q, 512)], oa[0:H + 1, :])
        else:
            nc.vector.tensor_add(
                outd[0:H + 1, ts(q, 512)], outd[0:H + 1, ts(q, 512)],
                oa[0:H + 1, :],
            )
        q_parts_done[q] += 1
        if q_parts_done[q] == q + 1:
            # quarter complete: ship it (bf16, 1KB contiguous per partition;
            # host divides by the denominator row and transposes)
            nc.sync.dma_start(out_d[:, ts(q, 512)], outd[:, ts(q, 512)])

    def flush(wid):
        # exp the filled window; then (while ACT runs) masks on GpSimd
        nonlocal pending
        if not pending:
            return
        wt, fill = win_tiles.pop(wid)
        assert fill == WBOUND[wid + 1] - WBOUND[wid], (wid, fill)
        pt0 = WBOUND[wid]
        nc.scalar.activation(pt_all[:, ds(pt0, fill)], wt[:, 0:fill], Exp,
                             scale=SCALE)
        for (j, t0, w, fpos) in pending:
            pt_off = pt0 + fpos
            # mask any part of this chunk inside the strip's diagonal block
            dlo, dhi = P * j, P * j + P
            mlo, mhi = max(t0, dlo), min(t0 + w, dhi)
            if mlo < mhi:
                nc.gpsimd.tensor_mul(
                    pt_all[:, ds(pt_off + (mlo - t0), mhi - mlo)],
                    pt_all[:, ds(pt_off + (mlo - t0), mhi - mlo)],
                    mask_sb[:, ds(mlo - dlo, mhi - mlo)],
                )
        pending = []

    emit_projqk(0)
    emit_projqk(1)
    emit_projqk(2)
    emit_projqk(3)
    emit_projv(0)

    # PE filler work slotted after specific windows: v-projections early
    # (they gate the v transposes), vtr batches just before the units that
    # consume them unlock
    fillers = {
        0: [lambda: emit_projv(1)],
        1: [lambda: emit_projv(2)],
        2: [lambda: emit_projv(3)] + [lambda j=j: emit_vtr(j)
                                      for j in range(0, 4)],
        3: [lambda j=j: emit_vtr(j) for j in range(4, 8)],
        4: [lambda j=j: emit_vtr(j) for j in range(8, 12)],
        5: [lambda j=j: emit_vtr(j) for j in range(12, 16)],
    }

    unit_i = 0

    def emit_ready_units(through_wid):
        # emit units whose unlock window has already been exp'd (two
        # windows behind the score stream, so the PE never waits on an
        # in-flight exp)
        nonlocal unit_i
        while unit_i < len(units) and units[unit_i][0] <= through_wid:
            _w, b, q = units[unit_i]
            emit_unit(b, q)
            unit_i += 1

    cur_wid = 0
    for (j, t0, w, fill) in all_chunks:
        wid = _wid_of(fill)
        fpos = fill - WBOUND[wid]
        if wid != cur_wid:
            flush(cur_wid)
            for f in fillers.get(cur_wid, ()):
                f()
            emit_ready_units(cur_wid - 1)
            cur_wid = wid
        if fpos == 0:
            wt = ps.tile([P, WIN], F32, tag="win", bufs=2, name=f"win{wid}")
            win_tiles[wid] = (wt, 0)
        wt, wfill = win_tiles[wid]
        assert wfill == fpos, (wfill, fpos)
        rg = (fill // 512) % 2
        stat = kA if rg == 0 else kB
        mov = qA if rg == 0 else qB
        nc.tensor.matmul(
            wt[:, ds(fpos, w)],
            stat[:, ds(P * j, P)],
            mov[:, ds(t0, w)],
            start=True, stop=True,
        )
        win_tiles[wid] = (wt, wfill + w)
        pending.append((j, t0, w, fpos))
    flush(cur_wid)
    emit_ready_units(N_WIN)
    assert unit_i == len(units), (unit_i, len(units))
    if dbg_d is not None:
        nc.sync.dma_start(dbg_d[:, 0:NT * (H + 1)],
                          v_sb.rearrange("p j h -> p (j h)"))
        nc.sync.dma_start(dbg_d[:, 2048:2048 + 4096],
                          pt_all[:, 0:4096])


def _build_program(num_devices=B, debug_out=False):
    nc = bacc.Bacc("TRN2", target_bir_lowering=False, debug=False,
                   num_devices=num_devices)
    xT_d = nc.dram_tensor("xT", [4, P, NCC, 512], BF16,
                          kind="ExternalInput").ap()
    wqk_d = nc.dram_tensor("wqk", [P, NCC, P], BF16, kind="ExternalInput").ap()
    wv_d = nc.dram_tensor("wv", [P, NCC, H], BF16, kind="ExternalInput").ap()
    mask_d = nc.dram_tensor("mask", [P, P], BF16, kind="ExternalInput").ap()
    ident_d = nc.dram_tensor("ident", [P, P], BF16, kind="ExternalInput").ap()
    out_d = nc.dram_tensor("out", [H + 1, T], BF16, kind="ExternalOutput").ap()
    dbg_d = None
    if debug_out:
        dbg_d = nc.dram_tensor("dbg", [P, 8192], BF16,
                               kind="ExternalOutput").ap()
    from contextlib import ExitStack

    with tile.TileContext(nc) as tc:
        with ExitStack() as ctx:
            _emit(tc, xT_d, wqk_d, wv_d, mask_d, ident_d,
                  out_d, ctx, dbg_d=dbg_d)
    nc.compile()
    return nc


def _host_inputs(x, Wq, Wk, Wv):
    bf = ml_dtypes.bfloat16
    xT = np.ascontiguousarray(np.transpose(x, (0, 2, 1))).astype(bf)
    Bn = x.shape[0]
    # xT: [t4, 128, c, 512] -- one contiguous run per (partition, t4)
    xTr = xT.reshape(Bn, NCC, P, 4, 512)
    xTn = np.ascontiguousarray(xTr.transpose(0, 3, 2, 1, 4))
    wqk = np.concatenate([Wq, Wk], axis=1).reshape(NCC, P, 2 * H)
    wqk = np.ascontiguousarray(np.transpose(wqk, (1, 0, 2))).astype(bf)
    wv = np.ascontiguousarray(
        np.transpose(Wv.reshape(NCC, P, H), (1, 0, 2))
    ).astype(bf)
    # mask[s, t] = 1 where s <= t (transposed-causal diagonal block)
    mask = np.triu(np.ones((P, P), dtype=np.float32)).astype(bf)
    identity = np.eye(P, dtype=np.float32).astype(bf)
    return xTn, wqk, wv, mask, identity


def kernel(x, Wq, Wk, Wv):
    global LAST_RESULT, _PROGRAM
    assert x.shape == (B, T, C), x.shape
    if _PROGRAM is None:
        _PROGRAM = _build_program()
    nc = _PROGRAM

    xTn, wqk, wv, mask, identity = _host_inputs(x, Wq, Wk, Wv)
    in_maps = [
        {"xT": xTn[b], "wqk": wqk, "wv": wv, "mask": mask, "ident": identity}
        for b in range(B)
    ]
    trace = bool(int(os.environ.get("KERNEL_TRACE", "0")))
    kw = {}
    td = os.environ.get("KERNEL_TRACE_DIR")
    if td:
        kw["tmpdir"] = td
    LAST_RESULT = run_bass_kernel_spmd(
        nc, in_maps, list(range(B)), trace=trace, **kw
    )
    out = np.empty((B, T, H), dtype=np.float32)
    for b in range(B):
        acc = LAST_RESULT.results[b]["out"].astype(np.float32)  # [65, T]
        out[b] = (acc[0:H] / acc[H:H + 1]).T
    return out
